# revision 1
# baseline (speedup 1.0000x reference)
"""DGCNN (nn_DGCNN_param_57904749085240) Trainium2 Bass kernel.

Data-parallel over batch: 8 cores x 2 point clouds each, no collectives.

Per EdgeConv layer, instead of materializing (2C, N, k) edge features, use
    W @ [x_j - x_i; x_i] = W1 x_j + (W2 - W1) x_i
and eval-BN + leaky folding (per-channel scale s > 0 commutes with max_k):
    y[:, i] = leaky( max_{j in knn(i)} (A x_j)  +  Cc x_i + t )
with A = s*W1, Cc = s*(W2-W1), t = s*b + beta - s*mu, all host-folded.

knn is exact fp32: pd = 2 X^T X - xx_i - xx_j comes out of the PE via an
augmented matmul ([X | ones | xx] pairs with [2X | -xx | -ones]); per
128-point tile the top-20 indices are 3 rounds of DVE max8/max_index/
match_replace.  Neighbor max = gpsimd ap_gather (SBUF->SBUF, fp32) with a
wrapped 16-partition index list + grouped DVE tensor_reduce max.
"""
import sys

sys.path.insert(0, "/opt/trn_rl_repo")

import numpy as np

import concourse.bacc as bacc
import concourse.tile as tile
from concourse import mybir
from concourse.bass_utils import run_bass_kernel_spmd

F32 = mybir.dt.float32
I16 = mybir.dt.int16
U16 = mybir.dt.uint16

B, N, K = 16, 1024, 20
N_CORES = 8
ELS = B // N_CORES
CH_C = [3, 64, 64, 128]
CH_O = [64, 64, 128, 256]
EMB = 1024
NT = N // 128
MMF = 512                     # fp32 matmul free-dim limit (one PSUM bank)
NEG = -1.0e30

AF = mybir.ActivationFunctionType
ALU = mybir.AluOpType
AX = mybir.AxisListType


def _mm(nc, out, lhsT, rhs, start, stop):
    fd = rhs.shape[-1]
    if fd <= MMF:
        nc.tensor.matmul(out=out, lhsT=lhsT, rhs=rhs, start=start, stop=stop)
        return
    for f0 in range(0, fd, MMF):
        f1 = min(f0 + MMF, fd)
        nc.tensor.matmul(out=out[:, f0:f1], lhsT=lhsT, rhs=rhs[:, f0:f1],
                         start=start, stop=stop)


def build_program(debug=False, reps=1, ablate=()):
    nc = bacc.Bacc("TRN2", target_bir_lowering=False, debug=False)

    x_in = nc.dram_tensor("x3", [ELS * 3, N], F32, kind="ExternalInput")
    wa_d, wc_d, wt_d = [], [], []
    for l in range(4):
        C, O = CH_C[l], CH_O[l]
        wa_d.append(nc.dram_tensor(f"wa{l}", [C, O], F32, kind="ExternalInput"))
        wc_d.append(nc.dram_tensor(f"wc{l}", [C, O], F32, kind="ExternalInput"))
        wt_d.append(nc.dram_tensor(f"wt{l}", [1, O], F32, kind="ExternalInput"))
    wlc_d = nc.dram_tensor("wlc", [513, EMB], F32, kind="ExternalInput")
    wl0_d = nc.dram_tensor("wl0", [2049, 512], F32, kind="ExternalInput")
    wl1_d = nc.dram_tensor("wl1", [513, 256], F32, kind="ExternalInput")
    wow_d = nc.dram_tensor("wow", [257, 40], F32, kind="ExternalInput")
    out_d = nc.dram_tensor("out", [ELS, 40], F32, kind="ExternalOutput")
    dbg = {}
    if debug:
        dbg["pd0"] = nc.dram_tensor("dbg_pd0", [128, N], F32, kind="ExternalOutput")
        dbg["idx0"] = nc.dram_tensor("dbg_idx0", [128, 24], F32, kind="ExternalOutput")
        dbg["m0"] = nc.dram_tensor("dbg_m0", [64, N], F32, kind="ExternalOutput")
        dbg["y0"] = nc.dram_tensor("dbg_y0", [64, N], F32, kind="ExternalOutput")
        dbg["h2"] = nc.dram_tensor("dbg_h2", [128, 16 * ELS], F32, kind="ExternalOutput")

    with tile.TileContext(nc) as tc:
        with (
            tc.tile_pool(name="w", bufs=1) as wpool,
            tc.tile_pool(name="y", bufs=1) as ypool,
            tc.tile_pool(name="s1", bufs=1) as spool1,
            tc.tile_pool(name="s", bufs=2) as spool,
            tc.tile_pool(name="pdp", bufs=5) as pdpool,
            tc.tile_pool(name="g", bufs=3) as gpool,
            tc.tile_pool(name="dr", bufs=2, space="DRAM") as dramp,
            tc.tile_pool(name="jit", bufs=3) as jitp,
        ):
            # ---------------- consts + resident weights ----------------
            ones_row = wpool.tile([1, N], F32, tag="ones_row")
            ones_col = wpool.tile([128, 1], F32, tag="ones_col")
            ones2 = wpool.tile([1, ELS], F32, tag="ones2")
            nc.vector.memset(ones_row[:], 1.0)
            nc.vector.memset(ones_col[:], 1.0)
            nc.vector.memset(ones2[:], 1.0)

            x0_tiles = []
            for el in range(ELS):
                t = ypool.tile([3, N], F32, tag=f"x0_{el}", name=f"x0_{el}")
                nc.sync.dma_start(t[:], x_in.ap()[el * 3:(el + 1) * 3, :])
                x0_tiles.append(t)

            wa, wc, wt = [], [], []
            for l in range(4):
                C, O = CH_C[l], CH_O[l]
                ta = wpool.tile([C, O], F32, tag=f"wa{l}")
                tcc = wpool.tile([C, O], F32, tag=f"wc{l}")
                tt = wpool.tile([1, O], F32, tag=f"wt{l}")
                nc.sync.dma_start(ta[:], wa_d[l].ap())
                nc.sync.dma_start(tcc[:], wc_d[l].ap())
                nc.sync.dma_start(tt[:], wt_d[l].ap())
                wa.append(ta); wc.append(tcc); wt.append(tt)
            lc_rows = [(0, 64), (64, 128), (128, 256), (256, 384), (384, 512), (512, 513)]

            # h_parts[l][el] = list of ([<=128, N] AP) feature chunks (lc concat order)
            h_parts = [[None] * ELS for _ in range(4)]
            maxes = ypool.tile([128, NT, ELS], F32, tag="maxes")
            sums = ypool.tile([128, NT, ELS], F32, tag="sums")
            aug_t, raug_t = [], []
            for el in range(ELS):
                a1 = wpool.tile([2, N], F32, tag=f"aug{el}", name=f"aug{el}")
                a2 = wpool.tile([2, N], F32, tag=f"raug{el}", name=f"raug{el}")
                nc.sync.dma_start(a1[1:2, :], ones_row[:])
                nc.sync.dma_start(a2[0:1, :], ones_row[:])
                aug_t.append(a1); raug_t.append(a2)

            for _rep in range(reps):
              Xf = [x0_tiles[el][:] for el in range(ELS)]
              with (
                tc.tile_pool(name=f"pspd{_rep}", bufs=3, space="PSUM") as pspd,
                tc.tile_pool(name=f"psmm{_rep}", bufs=1, space="PSUM") as psmm,
              ):
                # ================= EdgeConv layers =================
                for l in range(4):
                    C, O = CH_C[l], CH_O[l]
                    packed = (O == 64 and ELS == 2)
                    nch = 1 if packed else O // 128  # gather-channel chunks per el

                    # wrapped+replicated idx tiles
                    if packed:
                        iwt = [spool1.tile([128, NT * 160], I16, tag="iw0", name="iw")]
                    else:
                        iwt = [spool1.tile([128, NT * 160], I16, tag=f"iw{el}", name=f"iw{el}")
                               for el in range(ELS)]

                    for el in range(ELS):
                        xf = Xf[el]
                        iw = iwt[0] if packed else iwt[el]
                        p_base = 64 * el if packed else 0
                        nrep = 4 if packed else 8

                        xsq = spool1.tile([C, N], F32, tag="xsq")
                        nc.scalar.activation(out=xsq[:], in_=xf, func=AF.Square)
                        xx_ps = psmm.tile([1, N], F32, tag="mm")
                        _mm(nc, xx_ps[:], ones_col[0:C, :], xsq[:], True, True)
                        # pd = 2G + nxx_i*1 + 1*nxx_j: aug = [nxx; 1], raug = [1; nxx]
                        aug, raug = aug_t[el], raug_t[el]
                        nc.scalar.activation(out=aug[0:1, :], in_=xx_ps[:], func=AF.Copy,
                                             scale=-1.0)
                        nc.sync.dma_start(raug[1:2, :], aug[0:1, :])
                        rhsf = spool1.tile([C, N], F32, tag="rhsf")
                        nc.vector.tensor_scalar_mul(rhsf[:], xf, 2.0)

                        flat = dramp.tile([NT * 128, K], I16, tag="idxflat")
                        for t in range(NT):
                            pd_ps = pspd.tile([128, N], F32, tag="pd")
                            _mm(nc, pd_ps[:], xf[:, t * 128:(t + 1) * 128], rhsf[:],
                                True, False)
                            _mm(nc, pd_ps[:], aug[:, t * 128:(t + 1) * 128], raug[:],
                                False, True)
                            pd_sb = pdpool.tile([128, N], F32, tag="pdsb")
                            nc.scalar.activation(out=pd_sb[:], in_=pd_ps[:], func=AF.Copy)
                            if debug and l == 0 and el == 0 and t == 0:
                                nc.sync.dma_start(dbg["pd0"].ap(), pd_sb[:])
                            v = pdpool.tile([128, 24], F32, tag="v")
                            vi = pdpool.tile([128, 24], U16, tag="vi")
                            if "topk" in ablate:
                                nc.vector.memset(vi[:], 0)
                            el_dummy = "topk" in ablate
                            if not el_dummy:
                                nc.vector.max(out=v[:, 0:8], in_=pd_sb[:])
                            if not el_dummy:
                                nc.vector.max_index(out=vi[:, 0:8], in_max=v[:, 0:8],
                                                in_values=pd_sb[:])
                            if not el_dummy:
                                nc.vector.match_replace(out=pd_sb[:], in_to_replace=v[:, 0:8],
                                                    in_values=pd_sb[:], imm_value=NEG)
                            if not el_dummy:
                                nc.vector.max(out=v[:, 8:16], in_=pd_sb[:])
                            if not el_dummy:
                                nc.vector.max_index(out=vi[:, 8:16], in_max=v[:, 8:16],
                                                in_values=pd_sb[:])
                            if not el_dummy:
                                nc.vector.match_replace(out=pd_sb[:], in_to_replace=v[:, 8:16],
                                                    in_values=pd_sb[:], imm_value=NEG)
                            if not el_dummy:
                                nc.vector.max(out=v[:, 16:24], in_=pd_sb[:])
                            if not el_dummy:
                                nc.vector.max_index(out=vi[:, 16:24], in_max=v[:, 16:24],
                                                in_values=pd_sb[:])
                            if debug and l == 0 and el == 0 and t == 0:
                                vf = pdpool.tile([128, 24], F32, tag="vf")
                                nc.vector.tensor_copy(vf[:], vi[:])
                                nc.sync.dma_start(dbg["idx0"].ap(), vf[:])
                            # e-order dump -> DRAM, then wrapped([16,160]) via
                            # per-tile: e-order dump -> wrapped strided read
                            # -> log2 replication (pipelines under rounds)
                            c0, c1 = t * 160, (t + 1) * 160
                            if "idxdma" in ablate:
                                if t == 0:
                                    nc.vector.memset(iw[:, 0:NT * 160], 0)
                            else:
                                nc.sync.dma_start(flat[t * 128:(t + 1) * 128, :],
                                                  vi[:, 0:K].bitcast(I16))
                                src = (flat[t * 128:(t + 1) * 128, :]
                                       .rearrange("p r -> (p r)")
                                       .rearrange("(s w) -> w s", w=16))
                                nc.sync.dma_start(iw[p_base:p_base + 16, c0:c1], src)
                                blk = 16
                                while blk < 16 * nrep:
                                    nc.sync.dma_start(
                                        iw[p_base + blk:p_base + 2 * blk, c0:c1],
                                        iw[p_base:p_base + blk, c0:c1])
                                    blk *= 2

                    # ---- convs + gather + activation ----
                    if packed:
                        a_sb = spool.tile([128, N], F32, tag="asb")
                        c_sb = spool.tile([128, N], F32, tag="csb")
                        for el in range(ELS):
                            a_ps = psmm.tile([64, N], F32, tag="mm")
                            _mm(nc, a_ps[:], wa[l][:, 0:O], Xf[el], True, True)
                            nc.scalar.activation(out=a_sb[64 * el:64 * (el + 1), :],
                                                 in_=a_ps[:], func=AF.Copy)
                            c_ps = psmm.tile([64, N], F32, tag="mm")
                            _mm(nc, c_ps[:], wc[l][:, 0:O], Xf[el], True, False)
                            _mm(nc, c_ps[:], wt[l][:, 0:O], ones_row[:], False, True)
                            nc.scalar.activation(out=c_sb[64 * el:64 * (el + 1), :],
                                                 in_=c_ps[:], func=AF.Copy)
                        m_sb = spool.tile([128, N], F32, tag="msb")
                        for t in range(NT):
                            g = gpool.tile([128, 2560], F32, tag="gath")
                            if "gather" in ablate:
                                nc.vector.memset(g[:, 0:4], 0.0)
                            else:
                                nc.gpsimd.ap_gather(
                                    out_ap=g[:], in_ap=a_sb[:],
                                    idxs_ap=iwt[0][:, t * 160:(t + 1) * 160],
                                    channels=128, num_elems=N, d=1, num_idxs=2560)
                            nc.vector.tensor_reduce(
                                out=m_sb[:, t * 128:(t + 1) * 128],
                                in_=g[:].rearrange("p (i r) -> p i r", r=K),
                                axis=AX.X, op=ALU.max)
                        if debug and l == 0:
                            nc.sync.dma_start(dbg["m0"].ap(), m_sb[0:64, :])
                        u = spool.tile([128, N], F32, tag="u")
                        nc.vector.tensor_tensor(out=u[:], in0=m_sb[:], in1=c_sb[:],
                                                op=ALU.add)
                        zs = spool.tile([128, N], F32, tag="zs")
                        nc.vector.tensor_scalar_mul(zs[:], u[:], 0.2)
                        newX = []
                        for el in range(ELS):
                            yt = ypool.tile([64, N], F32, tag=f"y{l}_{el}",
                                            name=f"y{l}_{el}")
                            nc.vector.tensor_tensor(
                                out=yt[:], in0=u[64 * el:64 * (el + 1), :],
                                in1=zs[64 * el:64 * (el + 1), :], op=ALU.max)
                            h_parts[l][el] = [yt[:]]
                            newX.append(yt[:])
                        Xf = newX
                        if debug and l == 0:
                            nc.sync.dma_start(dbg["y0"].ap(), Xf[0])
                    else:
                        newX = [None] * ELS
                        for el in range(ELS):
                            ychunks = []
                            for ch in range(nch):
                                o0, o1 = ch * 128, (ch + 1) * 128
                                a_sb = spool.tile([128, N], F32, tag="asb")
                                a_ps = psmm.tile([128, N], F32, tag="mm")
                                _mm(nc, a_ps[:], wa[l][:, o0:o1], Xf[el], True, True)
                                nc.scalar.activation(out=a_sb[:], in_=a_ps[:],
                                                     func=AF.Copy)
                                c_ps = psmm.tile([128, N], F32, tag="mm")
                                _mm(nc, c_ps[:], wc[l][:, o0:o1], Xf[el], True, False)
                                _mm(nc, c_ps[:], wt[l][:, o0:o1], ones_row[:],
                                    False, True)
                                c_sb = spool.tile([128, N], F32, tag="csb")
                                nc.scalar.activation(out=c_sb[:], in_=c_ps[:],
                                                     func=AF.Copy)
                                m_sb = spool.tile([128, N], F32, tag="msb")
                                for t in range(NT):
                                    g = gpool.tile([128, 2560], F32, tag="gath")
                                    if "gather" in ablate:
                                        nc.vector.memset(g[:, 0:4], 0.0)
                                    else:
                                        nc.gpsimd.ap_gather(
                                            out_ap=g[:], in_ap=a_sb[:],
                                            idxs_ap=iwt[el][:, t * 160:(t + 1) * 160],
                                            channels=128, num_elems=N, d=1, num_idxs=2560)
                                    nc.vector.tensor_reduce(
                                        out=m_sb[:, t * 128:(t + 1) * 128],
                                        in_=g[:].rearrange("p (i r) -> p i r", r=K),
                                        axis=AX.X, op=ALU.max)
                                u = spool.tile([128, N], F32, tag="u")
                                nc.vector.tensor_tensor(out=u[:], in0=m_sb[:],
                                                        in1=c_sb[:], op=ALU.add)
                                zs = spool.tile([128, N], F32, tag="zs")
                                nc.vector.tensor_scalar_mul(zs[:], u[:], 0.2)
                                yt = ypool.tile([128, N], F32, tag=f"y{l}_{el}_{ch}")
                                nc.vector.tensor_tensor(out=yt[:], in0=u[:], in1=zs[:],
                                                        op=ALU.max)
                                ychunks.append(yt[:])
                            h_parts[l][el] = ychunks
                            if nch == 1:
                                newX[el] = ychunks[0]
                        if l < 3:
                            Xf = newX

                    # ================= lc conv + pooling =================
                scr = ypool.tile([128, N], F32, tag="scr")
                for el in range(ELS):
                    rhs_chunks = (h_parts[0][el] + h_parts[1][el] + h_parts[2][el]
                                  + h_parts[3][el] + [ones_row[:]])
                    for mt in range(8):
                        u_ps = pspd.tile([128, N], F32, tag="pd")
                        for kc in range(6):
                            r0, r1 = lc_rows[kc]
                            wj = jitp.tile([r1 - r0, 128], F32, tag="wjlc", name="wjlc")
                            nc.sync.dma_start(
                                wj[:], wlc_d.ap()[r0:r1, mt * 128:(mt + 1) * 128])
                            _mm(nc, u_ps[:], wj[:], rhs_chunks[kc], kc == 0, kc == 5)
                        zs = spool.tile([128, N], F32, tag="zs")
                        nc.vector.tensor_scalar_mul(zs[:], u_ps[:], 0.2)
                        y5 = spool.tile([128, N], F32, tag="y5")
                        nc.vector.tensor_tensor(out=y5[:], in0=u_ps[:], in1=zs[:],
                                                op=ALU.max)
                        nc.vector.tensor_reduce(out=maxes[:, mt:mt + 1, el:el + 1],
                                                in_=y5[:], axis=AX.X, op=ALU.max)
                        nc.scalar.activation(out=scr[:], in_=y5[:], func=AF.Copy,
                                             accum_out=sums[:, mt:mt + 1, el:el + 1])
              if debug:
                  h2dbg = spool.tile([128, 16 * ELS], F32, tag="h2dbg")
                  nc.vector.tensor_copy(
                      h2dbg[:].rearrange("p (a b) -> p a b", a=16)[:, 0:8, :], maxes[:])
                  nc.vector.tensor_copy(
                      h2dbg[:].rearrange("p (a b) -> p a b", a=16)[:, 8:16, :], sums[:])
                  nc.sync.dma_start(dbg["h2"].ap(), h2dbg[:])

              # ================= FC head (els together as F=ELS) =================
              with tc.tile_pool(name=f"psfc{_rep}", bufs=1, space="PSUM") as psfc:
                  l0ps = [psfc.tile([128, ELS], F32, tag=f"fc{mt}", name=f"fc{mt}") for mt in range(4)]
                  for kc in range(17):
                      r0, r1 = (kc * 128, (kc + 1) * 128) if kc < 16 else (2048, 2049)
                      wj = jitp.tile([r1 - r0, 512], F32, tag="wj0")
                      nc.sync.dma_start(wj[:], wl0_d.ap()[r0:r1, :])
                      if kc < 8:
                          rhs = maxes[:, kc:kc + 1, :].rearrange("p a b -> p (a b)")
                      elif kc < 16:
                          rhs = sums[:, kc - 8:kc - 7, :].rearrange("p a b -> p (a b)")
                      else:
                          rhs = ones2[:]
                      for mt in range(4):
                          nc.tensor.matmul(out=l0ps[mt][:],
                                           lhsT=wj[:, mt * 128:(mt + 1) * 128],
                                           rhs=rhs, start=kc == 0, stop=kc == 16)
                  y6 = ypool.tile([128, 4 * ELS], F32, tag="y6")
                  y6v = y6[:].rearrange("p (a b) -> p a b", a=4)
                  for mt in range(4):
                      u = spool.tile([128, ELS], F32, tag="fcu")
                      zs = spool.tile([128, ELS], F32, tag="fczs")
                      nc.vector.tensor_scalar_mul(zs[:], l0ps[mt][:], 0.2)
                      nc.vector.tensor_tensor(out=u[:], in0=l0ps[mt][:], in1=zs[:],
                                              op=ALU.max)
                      nc.vector.tensor_copy(y6v[:, mt:mt + 1, :],
                                            u[:].rearrange("p (a b) -> p a b", a=1))
                  l1ps = [psfc.tile([128, ELS], F32, tag=f"fd{mt}", name=f"fd{mt}") for mt in range(2)]
                  for kc in range(5):
                      r0, r1 = (kc * 128, (kc + 1) * 128) if kc < 4 else (512, 513)
                      wj = jitp.tile([r1 - r0, 256], F32, tag="wj1")
                      nc.sync.dma_start(wj[:], wl1_d.ap()[r0:r1, :])
                      rhs = (y6v[:, kc:kc + 1, :].rearrange("p a b -> p (a b)")
                             if kc < 4 else ones2[:])
                      for mt in range(2):
                          nc.tensor.matmul(out=l1ps[mt][:],
                                           lhsT=wj[:, mt * 128:(mt + 1) * 128],
                                           rhs=rhs, start=kc == 0, stop=kc == 4)
                  y7 = ypool.tile([128, 2 * ELS], F32, tag="y7")
                  y7v = y7[:].rearrange("p (a b) -> p a b", a=2)
                  for mt in range(2):
                      u = spool.tile([128, ELS], F32, tag="fcu")
                      zs = spool.tile([128, ELS], F32, tag="fczs")
                      nc.vector.tensor_scalar_mul(zs[:], l1ps[mt][:], 0.2)
                      nc.vector.tensor_tensor(out=u[:], in0=l1ps[mt][:], in1=zs[:],
                                              op=ALU.max)
                      nc.vector.tensor_copy(y7v[:, mt:mt + 1, :],
                                            u[:].rearrange("p (a b) -> p a b", a=1))
                  ops_ = psfc.tile([ELS, 40], F32, tag="fcout")
                  for kc in range(3):
                      if kc < 2:
                          lhsT = y7v[:, kc:kc + 1, :].rearrange("p a b -> p (a b)")
                          wj = jitp.tile([128, 40], F32, tag="wjo")
                          nc.sync.dma_start(wj[:], wow_d.ap()[kc * 128:(kc + 1) * 128, :])
                      else:
                          lhsT = ones2[:]
                          wj = jitp.tile([1, 40], F32, tag="wjob")
                          nc.sync.dma_start(wj[:], wow_d.ap()[256:257, :])
                      nc.tensor.matmul(out=ops_[:], lhsT=lhsT, rhs=wj[:],
                                       start=kc == 0, stop=kc == 2)
                  osb = spool.tile([ELS, 40], F32, tag="osb")
                  nc.scalar.activation(out=osb[:], in_=ops_[:], func=AF.Copy)
                  nc.sync.dma_start(out_d.ap(), osb[:])

    nc.compile()
    return nc


def _fold_weights(i):
    out = {}
    for l in range(4):
        C = CH_C[l]
        w = np.asarray(i[f"c{l}_w"], np.float64)
        b = np.asarray(i[f"c{l}_b"], np.float64)
        g = np.asarray(i[f"c{l}_g"], np.float64)
        be = np.asarray(i[f"c{l}_be"], np.float64)
        m = np.asarray(i[f"c{l}_m"], np.float64)
        v = np.asarray(i[f"c{l}_v"], np.float64)
        s = g / np.sqrt(v + 1e-5)
        w1, w2 = w[:, :C], w[:, C:]
        out[f"wa{l}"] = np.ascontiguousarray((s[:, None] * w1).T).astype(np.float32)
        out[f"wc{l}"] = np.ascontiguousarray((s[:, None] * (w2 - w1)).T).astype(np.float32)
        out[f"wt{l}"] = (s * b + be - s * m).astype(np.float32)[None, :]
    s = np.asarray(i["lc_g"], np.float64) / np.sqrt(np.asarray(i["lc_v"], np.float64) + 1e-5)
    t = s * np.asarray(i["lc_b"], np.float64) + np.asarray(i["lc_be"], np.float64) \
        - s * np.asarray(i["lc_m"], np.float64)
    out["wlc"] = np.ascontiguousarray(
        np.concatenate([(s[:, None] * np.asarray(i["lc_w"], np.float64)).T,
                        t[None, :]], 0)).astype(np.float32)
    s = np.asarray(i["l0_g"], np.float64) / np.sqrt(np.asarray(i["l0_v"], np.float64) + 1e-5)
    t = np.asarray(i["l0_be"], np.float64) - s * np.asarray(i["l0_m"], np.float64)
    w = s[:, None] * np.asarray(i["l0_w"], np.float64)
    w[:, 1024:] /= 1024.0
    out["wl0"] = np.ascontiguousarray(np.concatenate([w.T, t[None, :]], 0)).astype(np.float32)
    s = np.asarray(i["l1_g"], np.float64) / np.sqrt(np.asarray(i["l1_v"], np.float64) + 1e-5)
    t = s * np.asarray(i["l1_b"], np.float64) + np.asarray(i["l1_be"], np.float64) \
        - s * np.asarray(i["l1_m"], np.float64)
    out["wl1"] = np.ascontiguousarray(
        np.concatenate([(s[:, None] * np.asarray(i["l1_w"], np.float64)).T,
                        t[None, :]], 0)).astype(np.float32)
    out["wow"] = np.ascontiguousarray(
        np.concatenate([np.asarray(i["ow"], np.float32).T,
                        np.asarray(i["ob"], np.float32)[None, :]], 0))
    return out


_NC_CACHE = {}


def get_program(debug=False):
    if debug not in _NC_CACHE:
        _NC_CACHE[debug] = build_program(debug)
    return _NC_CACHE[debug]


def make_in_maps(inputs):
    folded = _fold_weights(inputs)
    x = np.asarray(inputs["x"], np.float32)
    in_maps = []
    for c in range(N_CORES):
        m = dict(folded)
        xs = x[c * ELS:(c + 1) * ELS]                       # (ELS, 1024, 3)
        m["x3"] = np.ascontiguousarray(
            xs.transpose(0, 2, 1).reshape(ELS * 3, N))
        in_maps.append(m)
    return in_maps


def kernel(**inputs) -> np.ndarray:
    nc = get_program(False)
    in_maps = make_in_maps(inputs)
    res = run_bass_kernel_spmd(nc, in_maps, list(range(N_CORES)))
    outs = [res.results[c]["out"] for c in range(N_CORES)]
    return np.concatenate(outs, 0).astype(np.float32)



# revision 10
# speedup vs baseline: 1.1681x; 1.1681x over previous
"""DGCNN (nn_DGCNN_param_57904749085240) Trainium2 Bass kernel.

Data-parallel over batch: 8 cores x 2 point clouds each, no collectives.

Per EdgeConv layer, instead of materializing (2C, N, k) edge features, use
    W @ [x_j - x_i; x_i] = W1 x_j + (W2 - W1) x_i
and eval-BN + leaky folding (per-channel scale s > 0 commutes with max_k):
    y[:, i] = leaky( max_{j in knn(i)} (A x_j)  +  Cc x_i + t )
with A = s*W1, Cc = s*(W2-W1), t = s*b + beta - s*mu, all host-folded.

knn is exact fp32: pd = 2 X^T X - xx_i - xx_j comes out of the PE via an
augmented matmul ([X | ones | xx] pairs with [2X | -xx | -ones]); per
128-point tile the top-20 indices are 3 rounds of DVE max8/max_index/
match_replace.  Neighbor max = gpsimd ap_gather (SBUF->SBUF, fp32) with a
wrapped 16-partition index list + grouped DVE tensor_reduce max.
"""
import sys

sys.path.insert(0, "/opt/trn_rl_repo")

import numpy as np

import concourse.bacc as bacc
import concourse.tile as tile
from concourse import mybir
from concourse.bass_utils import run_bass_kernel_spmd

F32 = mybir.dt.float32
F32R = mybir.dt.float32r
F16 = mybir.dt.float16
I16 = mybir.dt.int16
U16 = mybir.dt.uint16

B, N, K = 16, 1024, 20
N_CORES = 8
ELS = B // N_CORES
CH_C = [3, 64, 64, 128]
CH_O = [64, 64, 128, 256]
EMB = 1024
NT = N // 128
MMF = 512                     # fp32 matmul free-dim limit (one PSUM bank)
NEG = -1.0e30

AF = mybir.ActivationFunctionType
ALU = mybir.AluOpType
AX = mybir.AxisListType


def _mm(nc, out, lhsT, rhs, start, stop):
    if lhsT.dtype == F32 and rhs.dtype == F32:
        lhsT = lhsT.bitcast(F32R)
        rhs = rhs.bitcast(F32R)
    fd = rhs.shape[-1]
    if fd <= MMF:
        nc.tensor.matmul(out=out, lhsT=lhsT, rhs=rhs, start=start, stop=stop)
        return
    for f0 in range(0, fd, MMF):
        f1 = min(f0 + MMF, fd)
        nc.tensor.matmul(out=out[:, f0:f1], lhsT=lhsT, rhs=rhs[:, f0:f1],
                         start=start, stop=stop)


def build_program(debug=False, reps=1, ablate=()):
    nc = bacc.Bacc("TRN2", target_bir_lowering=False, debug=False)

    x_in = nc.dram_tensor("x3", [ELS * 3, N], F32, kind="ExternalInput")
    iota_in = nc.dram_tensor("iotak", [128, 2 * N], U16, kind="ExternalInput")
    wa_d, wc_d, wt_d = [], [], []
    for l in range(4):
        C, O = CH_C[l], CH_O[l]
        wa_d.append(nc.dram_tensor(f"wa{l}", [C, O], F32, kind="ExternalInput"))
        wc_d.append(nc.dram_tensor(f"wc{l}", [C, O], F32, kind="ExternalInput"))
        wt_d.append(nc.dram_tensor(f"wt{l}", [1, O], F32, kind="ExternalInput"))
    wlc_d = nc.dram_tensor("wlc", [513, EMB], F32, kind="ExternalInput")
    wl0_d = nc.dram_tensor("wl0", [2049, 512], F32, kind="ExternalInput")
    wl1_d = nc.dram_tensor("wl1", [513, 256], F32, kind="ExternalInput")
    wow_d = nc.dram_tensor("wow", [257, 40], F32, kind="ExternalInput")
    out_d = nc.dram_tensor("out", [ELS, 40], F32, kind="ExternalOutput")
    dbg = {}
    if debug:
        dbg["pd0"] = nc.dram_tensor("dbg_pd0", [128, N], F32, kind="ExternalOutput")
        dbg["idx0"] = nc.dram_tensor("dbg_idx0", [128, 24], F32, kind="ExternalOutput")
        dbg["m0"] = nc.dram_tensor("dbg_m0", [64, N], F32, kind="ExternalOutput")
        dbg["y0"] = nc.dram_tensor("dbg_y0", [64, N], F32, kind="ExternalOutput")
        dbg["h2"] = nc.dram_tensor("dbg_h2", [128, 16 * ELS], F32, kind="ExternalOutput")
    NKEYS = 3

    with tile.TileContext(nc) as tc:
        with (
            tc.tile_pool(name="w", bufs=1) as wpool,
            tc.tile_pool(name="y", bufs=1) as ypool,
            tc.tile_pool(name="s1", bufs=1) as spool1,
            tc.tile_pool(name="s", bufs=2) as spool,
            tc.tile_pool(name="pdp", bufs=5) as pdpool,
            tc.tile_pool(name="g", bufs=3) as gpool,
            tc.tile_pool(name="dr", bufs=2, space="DRAM") as dramp,
            tc.tile_pool(name="jit", bufs=3) as jitp,
        ):
            # ---------------- consts + resident weights ----------------
            ones_row = wpool.tile([1, N], F32, tag="ones_row")
            ones_col = wpool.tile([128, 1], F32, tag="ones_col")
            ones2 = wpool.tile([1, ELS], F32, tag="ones2")
            nc.vector.memset(ones_row[:], 1.0)
            nc.vector.memset(ones_col[:], 1.0)
            nc.vector.memset(ones2[:], 1.0)

            x0_tiles = []
            for el in range(ELS):
                t = ypool.tile([3, N], F32, tag=f"x0_{el}", name=f"x0_{el}")
                nc.sync.dma_start(t[:], x_in.ap()[el * 3:(el + 1) * 3, :])
                x0_tiles.append(t)

            # packed-key tiles: even u16 = column iota, odd u16 = fp16(q).
            # match_replace clobbers whole fp32 words (iota bits included), so
            # each reuse re-DMAs the iota const before the fp16 evac.
            keys_tiles = []
            for i in range(NKEYS):
                kt = wpool.tile([128, 2 * N], U16, tag=f"keys{i}", name=f"keys{i}")
                keys_tiles.append(kt)
            key_rot = [0]

            wa, wc, wt = [], [], []
            for l in range(4):
                C, O = CH_C[l], CH_O[l]
                ta = wpool.tile([C, O], F32, tag=f"wa{l}")
                tcc = wpool.tile([C, O], F32, tag=f"wc{l}")
                tt = wpool.tile([1, O], F32, tag=f"wt{l}")
                nc.sync.dma_start(ta[:], wa_d[l].ap())
                nc.sync.dma_start(tcc[:], wc_d[l].ap())
                nc.sync.dma_start(tt[:], wt_d[l].ap())
                wa.append(ta); wc.append(tcc); wt.append(tt)
            lc_rows = [(0, 64), (64, 128), (128, 256), (256, 384), (384, 512), (512, 513)]

            # h_parts[l][el] = list of ([<=128, N] AP) feature chunks (lc concat order)
            h_parts = [[None] * ELS for _ in range(4)]
            maxes = ypool.tile([128, NT, ELS], F32, tag="maxes")
            sums = ypool.tile([128, NT, ELS], F32, tag="sums")
            aug_t, raug_t = [], []
            for el in range(ELS):
                a1 = wpool.tile([2, N], F32, tag=f"aug{el}", name=f"aug{el}")
                a2 = wpool.tile([2, N], F32, tag=f"raug{el}", name=f"raug{el}")
                nc.sync.dma_start(a1[1:2, :], ones_row[:])
                nc.sync.dma_start(a2[0:1, :], ones_row[:])
                aug_t.append(a1); raug_t.append(a2)

            for _rep in range(reps):
              Xf = [x0_tiles[el][:] for el in range(ELS)]
              with (
                tc.tile_pool(name=f"pspd{_rep}", bufs=3, space="PSUM") as pspd,
                tc.tile_pool(name=f"psmm{_rep}", bufs=1, space="PSUM") as psmm,
              ):
                # ================= EdgeConv layers =================
                for l in range(4):
                    C, O = CH_C[l], CH_O[l]
                    packed = (O == 64 and ELS == 2)
                    nch = 1 if packed else O // 128  # gather-channel chunks per el

                    # wrapped+replicated idx tiles
                    if packed:
                        iwt = [spool1.tile([128, NT * 160], I16, tag="iw0", name="iw")]
                    else:
                        iwt = [spool1.tile([128, NT * 160], I16, tag=f"iw{el}", name=f"iw{el}")
                               for el in range(ELS)]

                    for el in range(ELS):
                        xf = Xf[el]
                        iw = iwt[0] if packed else iwt[el]
                        p_base = 64 * el if packed else 0
                        nrep = 4 if packed else 8

                        xsq = spool1.tile([C, N], F32, tag="xsq")
                        nc.scalar.activation(out=xsq[:], in_=xf, func=AF.Square)
                        xx_ps = psmm.tile([1, N], F32, tag="mm")
                        _mm(nc, xx_ps[:], ones_col[0:C, :], xsq[:], True, True)
                        # q = pd + 0.5: aug = [0.25-xx; 1], raug = [1; 0.25-xx]
                        aug, raug = aug_t[el], raug_t[el]
                        nc.scalar.activation(out=aug[0:1, :], in_=xx_ps[:], func=AF.Copy,
                                             scale=-1.0, bias=0.25)
                        nc.sync.dma_start(raug[1:2, :], aug[0:1, :])
                        rhsf = spool1.tile([C, N], F32, tag="rhsf")
                        nc.vector.tensor_scalar_mul(rhsf[:], xf, 2.0)

                        flat = dramp.tile([NT * 128, K], I16, tag="idxflat")
                        for t in range(NT):
                            pd_ps = pspd.tile([128, N], F32, tag="pd")
                            _mm(nc, pd_ps[:], xf[:, t * 128:(t + 1) * 128], rhsf[:],
                                True, False)
                            _mm(nc, pd_ps[:], aug[:, t * 128:(t + 1) * 128], raug[:],
                                False, True)
                            # evac PSUM -> odd u16 halves of the key tile (fp16 cast);
                            # even halves hold the column iota -> fp32 keys rank by
                            # (fp16 q, col) with col as tiebreak, idx free in low bits
                            kb = keys_tiles[key_rot[0] % NKEYS]
                            key_rot[0] += 1
                            nc.sync.dma_start(kb[:], iota_in.ap())
                            kodd = (kb[:].rearrange("p (n two) -> p n two", two=2)
                                    [:, :, 1:2].bitcast(F16))
                            nc.scalar.activation(out=kodd, in_=pd_ps[:], func=AF.Copy)
                            kv = kb[:].bitcast(F32)
                            if debug and l == 0 and el == 0 and t == 0:
                                nc.sync.dma_start(dbg["pd0"].ap(), kv)
                            v = pdpool.tile([128, 24], F32, tag="v")
                            el_dummy = "topk" in ablate
                            if el_dummy:
                                nc.vector.memset(v[:], 0)
                            if not el_dummy:
                                nc.vector.max(out=v[:, 0:8], in_=kv)
                            if not el_dummy:
                                nc.vector.match_replace(out=kv, in_to_replace=v[:, 0:8],
                                                    in_values=kv, imm_value=NEG)
                            if not el_dummy:
                                nc.vector.max(out=v[:, 8:16], in_=kv)
                            if not el_dummy:
                                nc.vector.match_replace(out=kv, in_to_replace=v[:, 8:16],
                                                    in_values=kv, imm_value=NEG)
                            if not el_dummy:
                                nc.vector.max(out=v[:, 16:24], in_=kv)
                            if debug and l == 0 and el == 0 and t == 0:
                                nc.sync.dma_start(dbg["idx0"].ap(), v[:])
                            # idx u16s sit at even half-words of v; strided dump
                            # -> DRAM, wrapped([16,160]) read, log2 replication
                            c0, c1 = t * 160, (t + 1) * 160
                            if "idxdma" in ablate:
                                if t == 0:
                                    nc.vector.memset(iw[:, 0:NT * 160], 0)
                            else:
                                vi_src = (v[:].bitcast(I16)
                                          .rearrange("p (k two) -> p k two", two=2)
                                          [:, 0:K, 0:1]
                                          .rearrange("p k one -> p (k one)"))
                                nc.sync.dma_start(flat[t * 128:(t + 1) * 128, :],
                                                  vi_src)
                                src = (flat[t * 128:(t + 1) * 128, :]
                                       .rearrange("p r -> (p r)")
                                       .rearrange("(s w) -> w s", w=16))
                                nc.sync.dma_start(iw[p_base:p_base + 16, c0:c1], src)
                                blk = 16
                                while blk < 16 * nrep:
                                    nc.sync.dma_start(
                                        iw[p_base + blk:p_base + 2 * blk, c0:c1],
                                        iw[p_base:p_base + blk, c0:c1])
                                    blk *= 2

                    # ---- convs + gather + activation ----
                    if packed:
                        a_sb = spool.tile([128, N], F32, tag="asb")
                        c_sb = spool.tile([128, N], F32, tag="csb")
                        for el in range(ELS):
                            a_ps = psmm.tile([64, N], F32, tag="mm")
                            _mm(nc, a_ps[:], wa[l][:, 0:O], Xf[el], True, True)
                            nc.scalar.activation(out=a_sb[64 * el:64 * (el + 1), :],
                                                 in_=a_ps[:], func=AF.Copy)
                            c_ps = psmm.tile([64, N], F32, tag="mm")
                            _mm(nc, c_ps[:], wc[l][:, 0:O], Xf[el], True, False)
                            _mm(nc, c_ps[:], wt[l][:, 0:O], ones_row[:], False, True)
                            nc.scalar.activation(out=c_sb[64 * el:64 * (el + 1), :],
                                                 in_=c_ps[:], func=AF.Copy)
                        m_sb = spool.tile([128, N], F32, tag="msb")
                        for t in range(NT):
                            g = gpool.tile([128, 2560], F32, tag="gath")
                            if "gather" in ablate:
                                nc.vector.memset(g[:, 0:4], 0.0)
                            else:
                                nc.gpsimd.ap_gather(
                                    out_ap=g[:], in_ap=a_sb[:],
                                    idxs_ap=iwt[0][:, t * 160:(t + 1) * 160],
                                    channels=128, num_elems=N, d=1, num_idxs=2560)
                            nc.vector.tensor_reduce(
                                out=m_sb[:, t * 128:(t + 1) * 128],
                                in_=g[:].rearrange("p (i r) -> p i r", r=K),
                                axis=AX.X, op=ALU.max)
                        if debug and l == 0:
                            nc.sync.dma_start(dbg["m0"].ap(), m_sb[0:64, :])
                        u = spool.tile([128, N], F32, tag="u")
                        nc.vector.tensor_tensor(out=u[:], in0=m_sb[:], in1=c_sb[:],
                                                op=ALU.add)
                        zs = spool.tile([128, N], F32, tag="zs")
                        nc.vector.tensor_scalar_mul(zs[:], u[:], 0.2)
                        newX = []
                        for el in range(ELS):
                            yt = ypool.tile([64, N], F32, tag=f"y{l}_{el}",
                                            name=f"y{l}_{el}")
                            nc.vector.tensor_tensor(
                                out=yt[:], in0=u[64 * el:64 * (el + 1), :],
                                in1=zs[64 * el:64 * (el + 1), :], op=ALU.max)
                            h_parts[l][el] = [yt[:]]
                            newX.append(yt[:])
                        Xf = newX
                        if debug and l == 0:
                            nc.sync.dma_start(dbg["y0"].ap(), Xf[0])
                    else:
                        newX = [None] * ELS
                        for el in range(ELS):
                            ychunks = []
                            for ch in range(nch):
                                o0, o1 = ch * 128, (ch + 1) * 128
                                a_sb = spool.tile([128, N], F32, tag="asb")
                                a_ps = psmm.tile([128, N], F32, tag="mm")
                                _mm(nc, a_ps[:], wa[l][:, o0:o1], Xf[el], True, True)
                                nc.scalar.activation(out=a_sb[:], in_=a_ps[:],
                                                     func=AF.Copy)
                                c_ps = psmm.tile([128, N], F32, tag="mm")
                                _mm(nc, c_ps[:], wc[l][:, o0:o1], Xf[el], True, False)
                                _mm(nc, c_ps[:], wt[l][:, o0:o1], ones_row[:],
                                    False, True)
                                c_sb = spool.tile([128, N], F32, tag="csb")
                                nc.scalar.activation(out=c_sb[:], in_=c_ps[:],
                                                     func=AF.Copy)
                                m_sb = spool.tile([128, N], F32, tag="msb")
                                for t in range(NT):
                                    g = gpool.tile([128, 2560], F32, tag="gath")
                                    if "gather" in ablate:
                                        nc.vector.memset(g[:, 0:4], 0.0)
                                    else:
                                        nc.gpsimd.ap_gather(
                                            out_ap=g[:], in_ap=a_sb[:],
                                            idxs_ap=iwt[el][:, t * 160:(t + 1) * 160],
                                            channels=128, num_elems=N, d=1, num_idxs=2560)
                                    nc.vector.tensor_reduce(
                                        out=m_sb[:, t * 128:(t + 1) * 128],
                                        in_=g[:].rearrange("p (i r) -> p i r", r=K),
                                        axis=AX.X, op=ALU.max)
                                u = spool.tile([128, N], F32, tag="u")
                                nc.vector.tensor_tensor(out=u[:], in0=m_sb[:],
                                                        in1=c_sb[:], op=ALU.add)
                                zs = spool.tile([128, N], F32, tag="zs")
                                nc.vector.tensor_scalar_mul(zs[:], u[:], 0.2)
                                yt = ypool.tile([128, N], F32, tag=f"y{l}_{el}_{ch}")
                                nc.vector.tensor_tensor(out=yt[:], in0=u[:], in1=zs[:],
                                                        op=ALU.max)
                                ychunks.append(yt[:])
                            h_parts[l][el] = ychunks
                            if nch == 1:
                                newX[el] = ychunks[0]
                        if l < 3:
                            Xf = newX

                    # ================= lc conv + pooling =================
                scr = ypool.tile([128, N], F32, tag="scr")
                for el in range(ELS):
                    rhs_chunks = (h_parts[0][el] + h_parts[1][el] + h_parts[2][el]
                                  + h_parts[3][el] + [ones_row[:]])
                    for mt in range(8):
                        u_ps = pspd.tile([128, N], F32, tag="pd")
                        for kc in range(6):
                            r0, r1 = lc_rows[kc]
                            wj = jitp.tile([r1 - r0, 128], F32, tag="wjlc", name="wjlc")
                            nc.sync.dma_start(
                                wj[:], wlc_d.ap()[r0:r1, mt * 128:(mt + 1) * 128])
                            _mm(nc, u_ps[:], wj[:], rhs_chunks[kc], kc == 0, kc == 5)
                        zs = spool.tile([128, N], F32, tag="zs")
                        nc.vector.tensor_scalar_mul(zs[:], u_ps[:], 0.2)
                        y5 = spool.tile([128, N], F32, tag="y5")
                        nc.vector.tensor_tensor(out=y5[:], in0=u_ps[:], in1=zs[:],
                                                op=ALU.max)
                        nc.vector.tensor_reduce(out=maxes[:, mt:mt + 1, el:el + 1],
                                                in_=y5[:], axis=AX.X, op=ALU.max)
                        nc.scalar.activation(out=scr[:], in_=y5[:], func=AF.Copy,
                                             accum_out=sums[:, mt:mt + 1, el:el + 1])
              if debug:
                  h2dbg = spool.tile([128, 16 * ELS], F32, tag="h2dbg")
                  nc.vector.tensor_copy(
                      h2dbg[:].rearrange("p (a b) -> p a b", a=16)[:, 0:8, :], maxes[:])
                  nc.vector.tensor_copy(
                      h2dbg[:].rearrange("p (a b) -> p a b", a=16)[:, 8:16, :], sums[:])
                  nc.sync.dma_start(dbg["h2"].ap(), h2dbg[:])

              # ================= FC head (els together as F=ELS) =================
              with tc.tile_pool(name=f"psfc{_rep}", bufs=1, space="PSUM") as psfc:
                  l0ps = [psfc.tile([128, ELS], F32, tag=f"fc{mt}", name=f"fc{mt}") for mt in range(4)]
                  for kc in range(17):
                      r0, r1 = (kc * 128, (kc + 1) * 128) if kc < 16 else (2048, 2049)
                      wj = jitp.tile([r1 - r0, 512], F32, tag="wj0")
                      nc.sync.dma_start(wj[:], wl0_d.ap()[r0:r1, :])
                      if kc < 8:
                          rhs = maxes[:, kc:kc + 1, :].rearrange("p a b -> p (a b)")
                      elif kc < 16:
                          rhs = sums[:, kc - 8:kc - 7, :].rearrange("p a b -> p (a b)")
                      else:
                          rhs = ones2[:]
                      for mt in range(4):
                          nc.tensor.matmul(out=l0ps[mt][:],
                                           lhsT=wj[:, mt * 128:(mt + 1) * 128],
                                           rhs=rhs, start=kc == 0, stop=kc == 16)
                  y6 = ypool.tile([128, 4 * ELS], F32, tag="y6")
                  y6v = y6[:].rearrange("p (a b) -> p a b", a=4)
                  for mt in range(4):
                      u = spool.tile([128, ELS], F32, tag="fcu")
                      zs = spool.tile([128, ELS], F32, tag="fczs")
                      nc.vector.tensor_scalar_mul(zs[:], l0ps[mt][:], 0.2)
                      nc.vector.tensor_tensor(out=u[:], in0=l0ps[mt][:], in1=zs[:],
                                              op=ALU.max)
                      nc.vector.tensor_copy(y6v[:, mt:mt + 1, :],
                                            u[:].rearrange("p (a b) -> p a b", a=1))
                  l1ps = [psfc.tile([128, ELS], F32, tag=f"fd{mt}", name=f"fd{mt}") for mt in range(2)]
                  for kc in range(5):
                      r0, r1 = (kc * 128, (kc + 1) * 128) if kc < 4 else (512, 513)
                      wj = jitp.tile([r1 - r0, 256], F32, tag="wj1")
                      nc.sync.dma_start(wj[:], wl1_d.ap()[r0:r1, :])
                      rhs = (y6v[:, kc:kc + 1, :].rearrange("p a b -> p (a b)")
                             if kc < 4 else ones2[:])
                      for mt in range(2):
                          nc.tensor.matmul(out=l1ps[mt][:],
                                           lhsT=wj[:, mt * 128:(mt + 1) * 128],
                                           rhs=rhs, start=kc == 0, stop=kc == 4)
                  y7 = ypool.tile([128, 2 * ELS], F32, tag="y7")
                  y7v = y7[:].rearrange("p (a b) -> p a b", a=2)
                  for mt in range(2):
                      u = spool.tile([128, ELS], F32, tag="fcu")
                      zs = spool.tile([128, ELS], F32, tag="fczs")
                      nc.vector.tensor_scalar_mul(zs[:], l1ps[mt][:], 0.2)
                      nc.vector.tensor_tensor(out=u[:], in0=l1ps[mt][:], in1=zs[:],
                                              op=ALU.max)
                      nc.vector.tensor_copy(y7v[:, mt:mt + 1, :],
                                            u[:].rearrange("p (a b) -> p a b", a=1))
                  ops_ = psfc.tile([ELS, 40], F32, tag="fcout")
                  for kc in range(3):
                      if kc < 2:
                          lhsT = y7v[:, kc:kc + 1, :].rearrange("p a b -> p (a b)")
                          wj = jitp.tile([128, 40], F32, tag="wjo")
                          nc.sync.dma_start(wj[:], wow_d.ap()[kc * 128:(kc + 1) * 128, :])
                      else:
                          lhsT = ones2[:]
                          wj = jitp.tile([1, 40], F32, tag="wjob")
                          nc.sync.dma_start(wj[:], wow_d.ap()[256:257, :])
                      nc.tensor.matmul(out=ops_[:], lhsT=lhsT, rhs=wj[:],
                                       start=kc == 0, stop=kc == 2)
                  osb = spool.tile([ELS, 40], F32, tag="osb")
                  nc.scalar.activation(out=osb[:], in_=ops_[:], func=AF.Copy)
                  nc.sync.dma_start(out_d.ap(), osb[:])

    nc.compile()
    return nc


def _fold_weights(i):
    out = {}
    for l in range(4):
        C = CH_C[l]
        w = np.asarray(i[f"c{l}_w"], np.float64)
        b = np.asarray(i[f"c{l}_b"], np.float64)
        g = np.asarray(i[f"c{l}_g"], np.float64)
        be = np.asarray(i[f"c{l}_be"], np.float64)
        m = np.asarray(i[f"c{l}_m"], np.float64)
        v = np.asarray(i[f"c{l}_v"], np.float64)
        s = g / np.sqrt(v + 1e-5)
        w1, w2 = w[:, :C], w[:, C:]
        out[f"wa{l}"] = np.ascontiguousarray((s[:, None] * w1).T).astype(np.float32)
        out[f"wc{l}"] = np.ascontiguousarray((s[:, None] * (w2 - w1)).T).astype(np.float32)
        out[f"wt{l}"] = (s * b + be - s * m).astype(np.float32)[None, :]
    s = np.asarray(i["lc_g"], np.float64) / np.sqrt(np.asarray(i["lc_v"], np.float64) + 1e-5)
    t = s * np.asarray(i["lc_b"], np.float64) + np.asarray(i["lc_be"], np.float64) \
        - s * np.asarray(i["lc_m"], np.float64)
    out["wlc"] = np.ascontiguousarray(
        np.concatenate([(s[:, None] * np.asarray(i["lc_w"], np.float64)).T,
                        t[None, :]], 0)).astype(np.float32)
    s = np.asarray(i["l0_g"], np.float64) / np.sqrt(np.asarray(i["l0_v"], np.float64) + 1e-5)
    t = np.asarray(i["l0_be"], np.float64) - s * np.asarray(i["l0_m"], np.float64)
    w = s[:, None] * np.asarray(i["l0_w"], np.float64)
    w[:, 1024:] /= 1024.0
    out["wl0"] = np.ascontiguousarray(np.concatenate([w.T, t[None, :]], 0)).astype(np.float32)
    s = np.asarray(i["l1_g"], np.float64) / np.sqrt(np.asarray(i["l1_v"], np.float64) + 1e-5)
    t = s * np.asarray(i["l1_b"], np.float64) + np.asarray(i["l1_be"], np.float64) \
        - s * np.asarray(i["l1_m"], np.float64)
    out["wl1"] = np.ascontiguousarray(
        np.concatenate([(s[:, None] * np.asarray(i["l1_w"], np.float64)).T,
                        t[None, :]], 0)).astype(np.float32)
    out["wow"] = np.ascontiguousarray(
        np.concatenate([np.asarray(i["ow"], np.float32).T,
                        np.asarray(i["ob"], np.float32)[None, :]], 0))
    return out


_NC_CACHE = {}


def get_program(debug=False):
    if debug not in _NC_CACHE:
        _NC_CACHE[debug] = build_program(debug)
    return _NC_CACHE[debug]


def make_in_maps(inputs):
    folded = _fold_weights(inputs)
    iotak = np.zeros((128, 2 * N), np.uint16)
    iotak[:, 0::2] = np.arange(N, dtype=np.uint16)[None, :]
    folded["iotak"] = iotak
    x = np.asarray(inputs["x"], np.float32)
    in_maps = []
    for c in range(N_CORES):
        m = dict(folded)
        xs = x[c * ELS:(c + 1) * ELS]                       # (ELS, 1024, 3)
        m["x3"] = np.ascontiguousarray(
            xs.transpose(0, 2, 1).reshape(ELS * 3, N))
        in_maps.append(m)
    return in_maps


def kernel(**inputs) -> np.ndarray:
    nc = get_program(False)
    in_maps = make_in_maps(inputs)
    res = run_bass_kernel_spmd(nc, in_maps, list(range(N_CORES)))
    outs = [res.results[c]["out"] for c in range(N_CORES)]
    return np.concatenate(outs, 0).astype(np.float32)



# revision 18
# speedup vs baseline: 1.3255x; 1.1348x over previous
"""DGCNN (nn_DGCNN_param_57904749085240) Trainium2 Bass kernel.

Data-parallel over batch: 8 cores x 2 point clouds each, no collectives.

Per EdgeConv layer, instead of materializing (2C, N, k) edge features, use
    W @ [x_j - x_i; x_i] = W1 x_j + (W2 - W1) x_i
and eval-BN + leaky folding (per-channel scale s > 0 commutes with max_k):
    y[:, i] = leaky( max_{j in knn(i)} (A x_j)  +  Cc x_i + t )
with A = s*W1, Cc = s*(W2-W1), t = s*b + beta - s*mu, all host-folded.

knn is exact fp32: pd = 2 X^T X - xx_i - xx_j comes out of the PE via an
augmented matmul ([X | ones | xx] pairs with [2X | -xx | -ones]); per
128-point tile the top-20 indices are 3 rounds of DVE max8/max_index/
match_replace.  Neighbor max = gpsimd ap_gather (SBUF->SBUF, fp32) with a
wrapped 16-partition index list + grouped DVE tensor_reduce max.
"""
import sys

sys.path.insert(0, "/opt/trn_rl_repo")

import numpy as np

import concourse.bacc as bacc
import concourse.tile as tile
from concourse import mybir
from concourse.bass_utils import run_bass_kernel_spmd

F32 = mybir.dt.float32
F32R = mybir.dt.float32r
F16 = mybir.dt.float16
I16 = mybir.dt.int16
U16 = mybir.dt.uint16

B, N, K = 16, 1024, 20
N_CORES = 8
ELS = B // N_CORES
CH_C = [3, 64, 64, 128]
CH_O = [64, 64, 128, 256]
EMB = 1024
NT = N // 128
MMF = 512                     # fp32 matmul free-dim limit (one PSUM bank)
NEG = -1.0e30

AF = mybir.ActivationFunctionType
ALU = mybir.AluOpType
AX = mybir.AxisListType


def _mm(nc, out, lhsT, rhs, start, stop):
    fd = rhs.shape[-1]
    if fd <= MMF:
        nc.tensor.matmul(out=out, lhsT=lhsT, rhs=rhs, start=start, stop=stop)
        return
    for f0 in range(0, fd, MMF):
        f1 = min(f0 + MMF, fd)
        nc.tensor.matmul(out=out[:, f0:f1], lhsT=lhsT, rhs=rhs[:, f0:f1],
                         start=start, stop=stop)


def build_program(debug=False, reps=1, ablate=()):
    nc = bacc.Bacc("TRN2", target_bir_lowering=False, debug=False)

    x_in = nc.dram_tensor("x3", [ELS * 3, N], F32R, kind="ExternalInput")
    iota_in = nc.dram_tensor("iotak", [128, 2 * N], U16, kind="ExternalInput")
    wa_d, wc_d, wt_d = [], [], []
    for l in range(4):
        C, O = CH_C[l], CH_O[l]
        wa_d.append(nc.dram_tensor(f"wa{l}", [C, O], F32R, kind="ExternalInput"))
        wc_d.append(nc.dram_tensor(f"wc{l}", [C, O], F32R, kind="ExternalInput"))
        wt_d.append(nc.dram_tensor(f"wt{l}", [1, O], F32R, kind="ExternalInput"))
    wlc_d = nc.dram_tensor("wlc", [513, EMB], F32R, kind="ExternalInput")
    wl0_d = nc.dram_tensor("wl0", [2049, 512], F32, kind="ExternalInput")
    wl1_d = nc.dram_tensor("wl1", [513, 256], F32, kind="ExternalInput")
    wow_d = nc.dram_tensor("wow", [257, 40], F32, kind="ExternalInput")
    out_d = nc.dram_tensor("out", [ELS, 40], F32, kind="ExternalOutput")
    dbg = {}
    if debug:
        dbg["pd0"] = nc.dram_tensor("dbg_pd0", [128, N], F32, kind="ExternalOutput")
        dbg["idx0"] = nc.dram_tensor("dbg_idx0", [128, 24], F32, kind="ExternalOutput")
        dbg["m0"] = nc.dram_tensor("dbg_m0", [64, N], F32, kind="ExternalOutput")
        dbg["y0"] = nc.dram_tensor("dbg_y0", [64, N], F32, kind="ExternalOutput")
        dbg["h2"] = nc.dram_tensor("dbg_h2", [128, 16 * ELS], F32, kind="ExternalOutput")
    NKEYS = 3

    with tile.TileContext(nc) as tc:
        with (
            tc.tile_pool(name="w", bufs=1) as wpool,
            tc.tile_pool(name="y", bufs=1) as ypool,
            tc.tile_pool(name="s1", bufs=1) as spool1,
            tc.tile_pool(name="s", bufs=2) as spool,
            tc.tile_pool(name="pdp", bufs=5) as pdpool,
            tc.tile_pool(name="g", bufs=3) as gpool,
            tc.tile_pool(name="dr", bufs=2, space="DRAM") as dramp,
            tc.tile_pool(name="jit", bufs=3) as jitp,
        ):
            # ---------------- consts + resident weights ----------------
            ones_f32 = wpool.tile([128, 1], F32, tag="ones_f32")
            ones_rf = wpool.tile([1, N], F32, tag="ones_rf")
            ones_row = wpool.tile([1, N], F32R, tag="ones_row")
            ones_col = wpool.tile([128, 1], F32R, tag="ones_col")
            ones2 = wpool.tile([1, ELS], F32, tag="ones2")
            nc.vector.memset(ones_f32[:], 1.0)
            nc.vector.memset(ones_rf[:], 1.0)
            nc.vector.memset(ones2[:], 1.0)
            # memset can't emit f32r (walrus ISA check); act-copy instead
            nc.scalar.activation(out=ones_row[:], in_=ones_rf[:], func=AF.Copy)
            nc.scalar.activation(out=ones_col[:], in_=ones_f32[:], func=AF.Copy)

            x0_tiles = []
            for el in range(ELS):
                t = ypool.tile([3, N], F32R, tag=f"x0_{el}", name=f"x0_{el}")
                nc.sync.dma_start(t[:], x_in.ap()[el * 3:(el + 1) * 3, :])
                x0_tiles.append(t)

            # packed-key tiles: even u16 = column iota, odd u16 = fp16(q).
            # match_replace clobbers whole fp32 words (iota bits included), so
            # each reuse re-DMAs the iota const before the fp16 evac.
            keys_tiles = []
            for i in range(NKEYS):
                kt = wpool.tile([128, 2 * N], U16, tag=f"keys{i}", name=f"keys{i}")
                keys_tiles.append(kt)
            key_rot = [0]

            wa, wc, wt = [], [], []
            for l in range(4):
                C, O = CH_C[l], CH_O[l]
                ta = wpool.tile([C, O], F32R, tag=f"wa{l}")
                tcc = wpool.tile([C, O], F32R, tag=f"wc{l}")
                tt = wpool.tile([1, O], F32R, tag=f"wt{l}")
                nc.sync.dma_start(ta[:], wa_d[l].ap())
                nc.sync.dma_start(tcc[:], wc_d[l].ap())
                nc.sync.dma_start(tt[:], wt_d[l].ap())
                wa.append(ta); wc.append(tcc); wt.append(tt)
            lc_rows = [(0, 64), (64, 128), (128, 256), (256, 384), (384, 512), (512, 513)]

            # h_parts[l][el] = list of ([<=128, N] AP) feature chunks (lc concat order)
            h_parts = [[None] * ELS for _ in range(4)]
            maxes = ypool.tile([128, NT, ELS], F32, tag="maxes")
            sums = ypool.tile([128, NT, ELS], F32, tag="sums")
            aug_t, raug_t = [], []
            for el in range(ELS):
                a1 = wpool.tile([2, N], F32R, tag=f"aug{el}", name=f"aug{el}")
                a2 = wpool.tile([2, N], F32R, tag=f"raug{el}", name=f"raug{el}")
                nc.sync.dma_start(a1[1:2, :], ones_row[:])
                nc.sync.dma_start(a2[0:1, :], ones_row[:])
                aug_t.append(a1); raug_t.append(a2)

            for _rep in range(reps):
              Xf = [x0_tiles[el][:] for el in range(ELS)]
              with (
                tc.tile_pool(name=f"pspd{_rep}", bufs=3, space="PSUM") as pspd,
                tc.tile_pool(name=f"psmm{_rep}", bufs=1, space="PSUM") as psmm,
              ):
                # ================= EdgeConv layers =================
                for l in range(4):
                    C, O = CH_C[l], CH_O[l]
                    packed = (O == 64 and ELS == 2)
                    nch = 1 if packed else O // 128  # gather-channel chunks per el

                    # wrapped+replicated idx tiles
                    if packed:
                        iwt = [spool1.tile([128, NT * 160], I16, tag="iw0", name="iw")]
                    else:
                        iwt = [spool1.tile([128, NT * 160], I16, tag=f"iw{el}", name=f"iw{el}")
                               for el in range(ELS)]

                    for el in range(ELS):
                        xf = Xf[el]
                        iw = iwt[0] if packed else iwt[el]
                        p_base = 64 * el if packed else 0
                        nrep = 4 if packed else 8

                        xsq = spool1.tile([C, N], F32R, tag="xsq")
                        nc.scalar.activation(out=xsq[:], in_=xf, func=AF.Square)
                        xx_ps = psmm.tile([1, N], F32, tag="mm")
                        _mm(nc, xx_ps[:], ones_col[0:C, :], xsq[:], True, True)
                        # q = pd + 0.5: aug = [0.25-xx; 1], raug = [1; 0.25-xx]
                        aug, raug = aug_t[el], raug_t[el]
                        nc.scalar.activation(out=aug[0:1, :], in_=xx_ps[:], func=AF.Copy,
                                             scale=-1.0, bias=0.25)
                        nc.sync.dma_start(raug[1:2, :], aug[0:1, :])
                        rhsf = spool1.tile([C, N], F32R, tag="rhsf")
                        nc.vector.tensor_scalar_mul(rhsf[:], xf, 2.0)

                        flat = dramp.tile([NT * 128, K], I16, tag="idxflat")
                        for t in range(NT):
                            pd_ps = pspd.tile([128, N], F32, tag="pd")
                            _mm(nc, pd_ps[:], xf[:, t * 128:(t + 1) * 128], rhsf[:],
                                True, False)
                            _mm(nc, pd_ps[:], aug[:, t * 128:(t + 1) * 128], raug[:],
                                False, True)
                            # evac PSUM -> odd u16 halves of the key tile (fp16 cast);
                            # even halves hold the column iota -> fp32 keys rank by
                            # (fp16 q, col) with col as tiebreak, idx free in low bits
                            kb = keys_tiles[key_rot[0] % NKEYS]
                            key_rot[0] += 1
                            nc.sync.dma_start(kb[:], iota_in.ap())
                            kodd = (kb[:].rearrange("p (n two) -> p n two", two=2)
                                    [:, :, 1:2].bitcast(F16))
                            nc.scalar.activation(out=kodd, in_=pd_ps[:], func=AF.Copy)
                            kv = kb[:].bitcast(F32)
                            if debug and l == 0 and el == 0 and t == 0:
                                nc.sync.dma_start(dbg["pd0"].ap(), kv)
                            v = pdpool.tile([128, 24], F32, tag="v")
                            el_dummy = "topk" in ablate
                            if el_dummy:
                                nc.vector.memset(v[:], 0)
                            if not el_dummy:
                                nc.vector.max(out=v[:, 0:8], in_=kv)
                            if not el_dummy:
                                nc.vector.match_replace(out=kv, in_to_replace=v[:, 0:8],
                                                    in_values=kv, imm_value=NEG)
                            if not el_dummy:
                                nc.vector.max(out=v[:, 8:16], in_=kv)
                            if not el_dummy:
                                nc.vector.match_replace(out=kv, in_to_replace=v[:, 8:16],
                                                    in_values=kv, imm_value=NEG)
                            if not el_dummy:
                                nc.vector.max(out=v[:, 16:24], in_=kv)
                            if debug and l == 0 and el == 0 and t == 0:
                                nc.sync.dma_start(dbg["idx0"].ap(), v[:])
                            # idx u16s sit at even half-words of v; strided dump
                            # -> DRAM, wrapped([16,160]) read, log2 replication
                            c0, c1 = t * 160, (t + 1) * 160
                            if "idxdma" in ablate:
                                if t == 0:
                                    nc.vector.memset(iw[:, 0:NT * 160], 0)
                            else:
                                vi_src = (v[:].bitcast(I16)
                                          .rearrange("p (k two) -> p k two", two=2)
                                          [:, 0:K, 0:1]
                                          .rearrange("p k one -> p (k one)"))
                                nc.sync.dma_start(flat[t * 128:(t + 1) * 128, :],
                                                  vi_src)
                                src = (flat[t * 128:(t + 1) * 128, :]
                                       .rearrange("p r -> (p r)")
                                       .rearrange("(s w) -> w s", w=16))
                                nc.sync.dma_start(iw[p_base:p_base + 16, c0:c1], src)
                                blk = 16
                                while blk < 16 * nrep:
                                    nc.sync.dma_start(
                                        iw[p_base + blk:p_base + 2 * blk, c0:c1],
                                        iw[p_base:p_base + blk, c0:c1])
                                    blk *= 2

                    # ---- convs + gather + activation ----
                    if packed:
                        a_sb = spool.tile([128, N], F32, tag="asb")
                        c_sb = spool.tile([128, N], F32, tag="csb")
                        for el in range(ELS):
                            a_ps = psmm.tile([64, N], F32, tag="mm")
                            _mm(nc, a_ps[:], wa[l][:, 0:O], Xf[el], True, True)
                            nc.scalar.activation(out=a_sb[64 * el:64 * (el + 1), :],
                                                 in_=a_ps[:], func=AF.Copy)
                            c_ps = psmm.tile([64, N], F32, tag="mm")
                            _mm(nc, c_ps[:], wc[l][:, 0:O], Xf[el], True, False)
                            _mm(nc, c_ps[:], wt[l][:, 0:O], ones_row[:], False, True)
                            nc.scalar.activation(out=c_sb[64 * el:64 * (el + 1), :],
                                                 in_=c_ps[:], func=AF.Copy)
                        m_sb = spool.tile([128, N], F32, tag="msb")
                        for t in range(NT):
                            g = gpool.tile([128, 2560], F32, tag="gath")
                            if "gather" in ablate:
                                nc.vector.memset(g[:, 0:4], 0.0)
                            else:
                                nc.gpsimd.ap_gather(
                                    out_ap=g[:], in_ap=a_sb[:],
                                    idxs_ap=iwt[0][:, t * 160:(t + 1) * 160],
                                    channels=128, num_elems=N, d=1, num_idxs=2560)
                            nc.vector.tensor_reduce(
                                out=m_sb[:, t * 128:(t + 1) * 128],
                                in_=g[:].rearrange("p (i r) -> p i r", r=K),
                                axis=AX.X, op=ALU.max)
                        if debug and l == 0:
                            nc.sync.dma_start(dbg["m0"].ap(), m_sb[0:64, :])
                        u = spool.tile([128, N], F32, tag="u")
                        nc.vector.tensor_tensor(out=u[:], in0=m_sb[:], in1=c_sb[:],
                                                op=ALU.add)
                        zs = spool.tile([128, N], F32, tag="zs")
                        nc.vector.tensor_scalar_mul(zs[:], u[:], 0.2)
                        newX = []
                        for el in range(ELS):
                            yt = ypool.tile([64, N], F32R, tag=f"y{l}_{el}",
                                            name=f"y{l}_{el}")
                            nc.vector.tensor_tensor(
                                out=yt[:], in0=u[64 * el:64 * (el + 1), :],
                                in1=zs[64 * el:64 * (el + 1), :], op=ALU.max)
                            h_parts[l][el] = [yt[:]]
                            newX.append(yt[:])
                        Xf = newX
                        if debug and l == 0:
                            nc.sync.dma_start(dbg["y0"].ap(), Xf[0])
                    else:
                        newX = [None] * ELS
                        for el in range(ELS):
                            ychunks = []
                            for ch in range(nch):
                                o0, o1 = ch * 128, (ch + 1) * 128
                                a_sb = spool.tile([128, N], F32, tag="asb")
                                a_ps = psmm.tile([128, N], F32, tag="mm")
                                _mm(nc, a_ps[:], wa[l][:, o0:o1], Xf[el], True, True)
                                nc.scalar.activation(out=a_sb[:], in_=a_ps[:],
                                                     func=AF.Copy)
                                c_ps = psmm.tile([128, N], F32, tag="mm")
                                _mm(nc, c_ps[:], wc[l][:, o0:o1], Xf[el], True, False)
                                _mm(nc, c_ps[:], wt[l][:, o0:o1], ones_row[:],
                                    False, True)
                                c_sb = spool.tile([128, N], F32, tag="csb")
                                nc.scalar.activation(out=c_sb[:], in_=c_ps[:],
                                                     func=AF.Copy)
                                m_sb = spool.tile([128, N], F32, tag="msb")
                                for t in range(NT):
                                    g = gpool.tile([128, 2560], F32, tag="gath")
                                    if "gather" in ablate:
                                        nc.vector.memset(g[:, 0:4], 0.0)
                                    else:
                                        nc.gpsimd.ap_gather(
                                            out_ap=g[:], in_ap=a_sb[:],
                                            idxs_ap=iwt[el][:, t * 160:(t + 1) * 160],
                                            channels=128, num_elems=N, d=1, num_idxs=2560)
                                    nc.vector.tensor_reduce(
                                        out=m_sb[:, t * 128:(t + 1) * 128],
                                        in_=g[:].rearrange("p (i r) -> p i r", r=K),
                                        axis=AX.X, op=ALU.max)
                                u = spool.tile([128, N], F32, tag="u")
                                nc.vector.tensor_tensor(out=u[:], in0=m_sb[:],
                                                        in1=c_sb[:], op=ALU.add)
                                zs = spool.tile([128, N], F32, tag="zs")
                                nc.vector.tensor_scalar_mul(zs[:], u[:], 0.2)
                                yt = ypool.tile([128, N], F32R, tag=f"y{l}_{el}_{ch}")
                                nc.vector.tensor_tensor(out=yt[:], in0=u[:], in1=zs[:],
                                                        op=ALU.max)
                                ychunks.append(yt[:])
                            h_parts[l][el] = ychunks
                            if nch == 1:
                                newX[el] = ychunks[0]
                        if l < 3:
                            Xf = newX

                    # ================= lc conv + pooling =================
                scr = ypool.tile([128, N], F32, tag="scr")
                for el in range(ELS):
                    rhs_chunks = (h_parts[0][el] + h_parts[1][el] + h_parts[2][el]
                                  + h_parts[3][el] + [ones_row[:]])
                    for mt in range(8):
                        u_ps = pspd.tile([128, N], F32, tag="pd")
                        for kc in range(6):
                            r0, r1 = lc_rows[kc]
                            wj = jitp.tile([r1 - r0, 128], F32R, tag="wjlc", name="wjlc")
                            nc.sync.dma_start(
                                wj[:], wlc_d.ap()[r0:r1, mt * 128:(mt + 1) * 128])
                            _mm(nc, u_ps[:], wj[:], rhs_chunks[kc], kc == 0, kc == 5)
                        zs = spool.tile([128, N], F32, tag="zs")
                        nc.vector.tensor_scalar_mul(zs[:], u_ps[:], 0.2)
                        y5 = spool.tile([128, N], F32, tag="y5")
                        nc.vector.tensor_tensor(out=y5[:], in0=u_ps[:], in1=zs[:],
                                                op=ALU.max)
                        nc.vector.tensor_reduce(out=maxes[:, mt:mt + 1, el:el + 1],
                                                in_=y5[:], axis=AX.X, op=ALU.max)
                        nc.scalar.activation(out=scr[:], in_=y5[:], func=AF.Copy,
                                             accum_out=sums[:, mt:mt + 1, el:el + 1])
              if debug:
                  h2dbg = spool.tile([128, 16 * ELS], F32, tag="h2dbg")
                  nc.vector.tensor_copy(
                      h2dbg[:].rearrange("p (a b) -> p a b", a=16)[:, 0:8, :], maxes[:])
                  nc.vector.tensor_copy(
                      h2dbg[:].rearrange("p (a b) -> p a b", a=16)[:, 8:16, :], sums[:])
                  nc.sync.dma_start(dbg["h2"].ap(), h2dbg[:])

              # ================= FC head (els together as F=ELS) =================
              with tc.tile_pool(name=f"psfc{_rep}", bufs=1, space="PSUM") as psfc:
                  l0ps = [psfc.tile([128, ELS], F32, tag=f"fc{mt}", name=f"fc{mt}") for mt in range(4)]
                  for kc in range(17):
                      r0, r1 = (kc * 128, (kc + 1) * 128) if kc < 16 else (2048, 2049)
                      wj = jitp.tile([r1 - r0, 512], F32, tag="wj0")
                      nc.sync.dma_start(wj[:], wl0_d.ap()[r0:r1, :])
                      if kc < 8:
                          rhs = maxes[:, kc:kc + 1, :].rearrange("p a b -> p (a b)")
                      elif kc < 16:
                          rhs = sums[:, kc - 8:kc - 7, :].rearrange("p a b -> p (a b)")
                      else:
                          rhs = ones2[:]
                      for mt in range(4):
                          nc.tensor.matmul(out=l0ps[mt][:],
                                           lhsT=wj[:, mt * 128:(mt + 1) * 128],
                                           rhs=rhs, start=kc == 0, stop=kc == 16)
                  y6 = ypool.tile([128, 4 * ELS], F32, tag="y6")
                  y6v = y6[:].rearrange("p (a b) -> p a b", a=4)
                  for mt in range(4):
                      u = spool.tile([128, ELS], F32, tag="fcu")
                      zs = spool.tile([128, ELS], F32, tag="fczs")
                      nc.vector.tensor_scalar_mul(zs[:], l0ps[mt][:], 0.2)
                      nc.vector.tensor_tensor(out=u[:], in0=l0ps[mt][:], in1=zs[:],
                                              op=ALU.max)
                      nc.vector.tensor_copy(y6v[:, mt:mt + 1, :],
                                            u[:].rearrange("p (a b) -> p a b", a=1))
                  l1ps = [psfc.tile([128, ELS], F32, tag=f"fd{mt}", name=f"fd{mt}") for mt in range(2)]
                  for kc in range(5):
                      r0, r1 = (kc * 128, (kc + 1) * 128) if kc < 4 else (512, 513)
                      wj = jitp.tile([r1 - r0, 256], F32, tag="wj1")
                      nc.sync.dma_start(wj[:], wl1_d.ap()[r0:r1, :])
                      rhs = (y6v[:, kc:kc + 1, :].rearrange("p a b -> p (a b)")
                             if kc < 4 else ones2[:])
                      for mt in range(2):
                          nc.tensor.matmul(out=l1ps[mt][:],
                                           lhsT=wj[:, mt * 128:(mt + 1) * 128],
                                           rhs=rhs, start=kc == 0, stop=kc == 4)
                  y7 = ypool.tile([128, 2 * ELS], F32, tag="y7")
                  y7v = y7[:].rearrange("p (a b) -> p a b", a=2)
                  for mt in range(2):
                      u = spool.tile([128, ELS], F32, tag="fcu")
                      zs = spool.tile([128, ELS], F32, tag="fczs")
                      nc.vector.tensor_scalar_mul(zs[:], l1ps[mt][:], 0.2)
                      nc.vector.tensor_tensor(out=u[:], in0=l1ps[mt][:], in1=zs[:],
                                              op=ALU.max)
                      nc.vector.tensor_copy(y7v[:, mt:mt + 1, :],
                                            u[:].rearrange("p (a b) -> p a b", a=1))
                  ops_ = psfc.tile([ELS, 40], F32, tag="fcout")
                  for kc in range(3):
                      if kc < 2:
                          lhsT = y7v[:, kc:kc + 1, :].rearrange("p a b -> p (a b)")
                          wj = jitp.tile([128, 40], F32, tag="wjo")
                          nc.sync.dma_start(wj[:], wow_d.ap()[kc * 128:(kc + 1) * 128, :])
                      else:
                          lhsT = ones2[:]
                          wj = jitp.tile([1, 40], F32, tag="wjob")
                          nc.sync.dma_start(wj[:], wow_d.ap()[256:257, :])
                      nc.tensor.matmul(out=ops_[:], lhsT=lhsT, rhs=wj[:],
                                       start=kc == 0, stop=kc == 2)
                  osb = spool.tile([ELS, 40], F32, tag="osb")
                  nc.scalar.activation(out=osb[:], in_=ops_[:], func=AF.Copy)
                  nc.sync.dma_start(out_d.ap(), osb[:])

    nc.compile()
    return nc


def _fold_weights(i):
    out = {}
    for l in range(4):
        C = CH_C[l]
        w = np.asarray(i[f"c{l}_w"], np.float64)
        b = np.asarray(i[f"c{l}_b"], np.float64)
        g = np.asarray(i[f"c{l}_g"], np.float64)
        be = np.asarray(i[f"c{l}_be"], np.float64)
        m = np.asarray(i[f"c{l}_m"], np.float64)
        v = np.asarray(i[f"c{l}_v"], np.float64)
        s = g / np.sqrt(v + 1e-5)
        w1, w2 = w[:, :C], w[:, C:]
        out[f"wa{l}"] = np.ascontiguousarray((s[:, None] * w1).T).astype(np.float32)
        out[f"wc{l}"] = np.ascontiguousarray((s[:, None] * (w2 - w1)).T).astype(np.float32)
        out[f"wt{l}"] = (s * b + be - s * m).astype(np.float32)[None, :]
    s = np.asarray(i["lc_g"], np.float64) / np.sqrt(np.asarray(i["lc_v"], np.float64) + 1e-5)
    t = s * np.asarray(i["lc_b"], np.float64) + np.asarray(i["lc_be"], np.float64) \
        - s * np.asarray(i["lc_m"], np.float64)
    out["wlc"] = np.ascontiguousarray(
        np.concatenate([(s[:, None] * np.asarray(i["lc_w"], np.float64)).T,
                        t[None, :]], 0)).astype(np.float32)
    s = np.asarray(i["l0_g"], np.float64) / np.sqrt(np.asarray(i["l0_v"], np.float64) + 1e-5)
    t = np.asarray(i["l0_be"], np.float64) - s * np.asarray(i["l0_m"], np.float64)
    w = s[:, None] * np.asarray(i["l0_w"], np.float64)
    w[:, 1024:] /= 1024.0
    out["wl0"] = np.ascontiguousarray(np.concatenate([w.T, t[None, :]], 0)).astype(np.float32)
    s = np.asarray(i["l1_g"], np.float64) / np.sqrt(np.asarray(i["l1_v"], np.float64) + 1e-5)
    t = s * np.asarray(i["l1_b"], np.float64) + np.asarray(i["l1_be"], np.float64) \
        - s * np.asarray(i["l1_m"], np.float64)
    out["wl1"] = np.ascontiguousarray(
        np.concatenate([(s[:, None] * np.asarray(i["l1_w"], np.float64)).T,
                        t[None, :]], 0)).astype(np.float32)
    out["wow"] = np.ascontiguousarray(
        np.concatenate([np.asarray(i["ow"], np.float32).T,
                        np.asarray(i["ob"], np.float32)[None, :]], 0))
    return out


_NC_CACHE = {}


def get_program(debug=False):
    if debug not in _NC_CACHE:
        _NC_CACHE[debug] = build_program(debug)
    return _NC_CACHE[debug]


def make_in_maps(inputs):
    folded = _fold_weights(inputs)
    iotak = np.zeros((128, 2 * N), np.uint16)
    iotak[:, 0::2] = np.arange(N, dtype=np.uint16)[None, :]
    folded["iotak"] = iotak
    x = np.asarray(inputs["x"], np.float32)
    in_maps = []
    for c in range(N_CORES):
        m = dict(folded)
        xs = x[c * ELS:(c + 1) * ELS]                       # (ELS, 1024, 3)
        m["x3"] = np.ascontiguousarray(
            xs.transpose(0, 2, 1).reshape(ELS * 3, N))
        in_maps.append(m)
    return in_maps


def kernel(**inputs) -> np.ndarray:
    nc = get_program(False)
    in_maps = make_in_maps(inputs)
    res = run_bass_kernel_spmd(nc, in_maps, list(range(N_CORES)))
    outs = [res.results[c]["out"] for c in range(N_CORES)]
    return np.concatenate(outs, 0).astype(np.float32)

